# revision 42
# baseline (speedup 1.0000x reference)
"""Trainium2 Bass kernel for nn_FDM_46394236731667.

Computes, per batch b (b = 0..7, one NeuronCore each):
    f1,f2,f3 = fm{1,2,3}[b].reshape(C, HW)
    qn  = f1 / max(||f1||_col, eps)  (column-wise L2 over channels)
    s_k = -(qn^T @ (f_k / max(||f_k||_col, eps)))          k in {2,3}
    a_k = softmax(s_k, axis=-1)
    out[b] = f1 + 0.001 * (f2 @ a2^T + f3 @ a3^T)

Implementation notes:
  - Scores are computed TRANSPOSED (tiles [m_partition, n_free]) so that the
    key-norm scale r_k[m] is a per-partition scalar folded into the ACT exp
    instruction (out = exp(scale[p] * psum)), the softmax denominator is a
    partition-reduction done with an all-ones matmul (result arrives already
    broadcast across partitions), and the P@V matmul consumes the exp tiles
    directly with no transposes of probability tiles.
  - Query norms r1[n] ride on the free axis, so they are pre-folded into the
    query operand qn = 16 * f1 * broadcast(r1) (the 16 makes fp8 quantization
    of unit-norm entries land in the normal range; the matching 1/16 is folded
    into the exp scale).
  - All three big matmul families (scores, softmax-denominator, values) run in
    fp8(e4m3) with perf_mode=DoubleRow: operands are stored with the
    contraction-chunk index as a middle AP dim so each matmul contracts 256
    rows. The output is fm1 + 0.001*(attention terms), so low-precision
    attention arithmetic perturbs the output by ~1e-6 relative.
  - Softmax skips the max-subtraction: scores are cosine similarities in
    [-1, 1], so exp() cannot overflow.
"""
import os
import sys

for _p in ("/opt/trn_rl_repo", "/root/.axon_site/_ro/trn_rl_repo"):
    if os.path.isdir(_p) and _p not in sys.path:
        sys.path.insert(0, _p)

import numpy as np

import concourse.bass as bass
import concourse.tile as tile
from concourse import bacc, mybir
from concourse.bass_utils import run_bass_kernel_spmd
from concourse.masks import make_identity

B, C, H, W = 8, 512, 56, 56
HW = H * W            # 3136
P = 128
CC = C // P           # 4 channel chunks
NMC = 25              # m chunks: 24 x 128 + 1 x 64
MTAIL = HW - 24 * P   # 64
NNC = 7               # n chunks
NW = HW // NNC        # 448
EPS = 1e-12
FACTOR = 0.001
QSCALE = 16.0         # fp8 headroom scale on qn; 1/QSCALE folded into exp

dt = mybir.dt
F32, BF16, FP8 = dt.float32, dt.bfloat16, dt.float8e4
DR = mybir.MatmulPerfMode.DoubleRow

TRACE = False
_cached_nc = None


def _mw(mc):
    return P if mc < NMC - 1 else MTAIL


def _build_preproc(tc, sbP, pre, ps, fm1, fm2, fm3):
    """sbP: persistent operand pool; pre: preproc transients (released before
    the main loop so its SBUF is reused for the main-loop pool)."""
    nc = tc.nc

    # ---- constants ----
    # fp8e5 identity: transposes run on e4m3 data bit-reinterpreted as e5m2
    # (pure data movement; our data never hits the e5m2 Inf/NaN encodings)
    ident = sbP.tile([P, P], dt.float8e3, tag="ident", name="ident")
    make_identity(nc, ident)
    ones128 = sbP.tile([P, 2, P], FP8, tag="ones128", name="ones128")
    nc.vector.memset(ones128, 1.0)
    ones_col = sbP.tile([P, 1], FP8, tag="ones_col", name="ones_col")
    nc.vector.memset(ones_col, 1.0)

    # =======================================================================
    # f1 phase: qn = QSCALE * f1 * broadcast(1/max(||f1||, eps)), fp8,
    # stored as [P, CC, HW] so score matmuls can take [P, 2, NW] slices.
    # =======================================================================
    # f1 squares are pre-scaled by 1/QSCALE so the resulting reciprocal norm
    # comes out as QSCALE/||f1|| with no extra pass.
    fsq1 = []
    fr1b = []
    for cc in range(CC):
        fr = pre.tile([P, HW], F32, tag="fraw", bufs=8, name=f"f1raw_{cc}")
        nc.sync.dma_start(fr, fm1[cc * P:(cc + 1) * P, :])
        t8 = pre.tile([P, HW], FP8, tag="fsq", bufs=4, name=f"fsq1_{cc}")
        nc.scalar.activation(t8, fr, mybir.ActivationFunctionType.Square,
                             bias=0.0, scale=1.0 / QSCALE)
        fsq1.append(t8)
        fr1b.append(fr)

    # rbf[p, n] = QSCALE/max(||f1[:,n]||, eps); qn produced per n-chunk so the
    # first score matmuls can start before the whole f1 phase finishes.
    rbf = pre.tile([P, HW], F32, tag="rbf", name="rbf")
    qn = sbP.tile([P, CC, HW], FP8, tag="qn", name="qn")
    for j in range(NNC):
        js = slice(j * NW, (j + 1) * NW)
        ssb = ps.tile([P, NW], F32, tag="cs", bufs=2, name=f"ss1b_{j}")
        for cc in range(CC):
            nc.tensor.matmul(ssb, ones128[:, 0, :], fsq1[cc][:, js],
                             start=(cc == 0), stop=(cc == CC - 1))
        ns = pre.tile([P, NW], F32, tag="rtmp", bufs=2, name=f"ns1_{j}")
        nc.scalar.sqrt(ns, ssb)
        nc.vector.tensor_scalar_max(ns, ns, EPS / QSCALE)
        nc.vector.reciprocal_approx_fast(rbf[:, js], ns)
        for cc in range(CC):
            nc.vector.tensor_mul(qn[:, cc, js], fr1b[cc][:, js], rbf[:, js])

    # =======================================================================
    # f2/f3 phases: fp8 key operand [P, CC, HW], fp8 transpose [P, NMC, C],
    # per-column key norms (as [P, NMC] columns for the exp scale)
    # =======================================================================
    def key_norms(fm, label):
        fb = sbP.tile([P, CC, HW], FP8, tag=f"{label}b", name=f"{label}b")
        fT = sbP.tile([P, NMC, C], FP8, tag=f"{label}T", name=f"{label}T")
        fsq = []
        for cc in range(CC):
            fr = pre.tile([P, HW], F32, tag="fraw", bufs=8,
                          name=f"{label}raw_{cc}")
            nc.sync.dma_start(fr, fm[cc * P:(cc + 1) * P, :])
            nc.vector.tensor_copy(fb[:, cc, :], fr)
            t8 = pre.tile([P, HW], FP8, tag="fsq", bufs=4,
                          name=f"{label}sq_{cc}")
            nc.scalar.square(t8, fr)
            fsq.append(t8)

        # ss columns: ssc[:mw, mc] = sum_c f[c, mc*128+p]^2
        ssc = ps.tile([P, NMC], F32, tag="cs", bufs=2, name=f"ssc_{label}")
        for mc in range(NMC):
            mw = _mw(mc)
            msl = slice(mc * P, mc * P + mw)
            for cc in range(CC):
                nc.tensor.matmul(ssc[:mw, mc:mc + 1], fsq[cc][:, msl], ones_col,
                                 start=(cc == 0), stop=(cc == CC - 1))
        nrm = pre.tile([P, NMC], F32, tag="rtmp2", bufs=2, name=f"nrm_{label}")
        nc.scalar.sqrt(nrm, ssc)
        nc.vector.tensor_scalar_max(nrm, nrm, EPS)
        rcp = pre.tile([P, NMC], F32, tag="rtmp3", bufs=2, name=f"rcp_{label}")
        nc.vector.reciprocal_approx_fast(rcp, nrm)
        rneg = sbP.tile([P, NMC], F32, tag=f"rneg_{label}", name=f"rneg_{label}")
        nc.vector.tensor_scalar_mul(rneg, rcp, -1.0 / QSCALE)
        return fb, fT, rneg

    E3 = dt.float8e3

    def transpose_one(fT, fb, label, cc, mc):
        # fT[p, mc, c] = f[c, mc*128+p]; PE transpose of the fp8 key operand
        # (bytes viewed as e5m2). The tp psum tiles share the "vp" tag: the
        # first value-matmul psum needs the transposes finished anyway, while
        # the score psum rotation ("sp" tag) stays free of them.
        mw = _mw(mc)
        msl = slice(mc * P, mc * P + mw)
        # fp8 transpose mode writes psum with element step 2
        tp = ps.tile([P, 2 * P], E3, tag="vp", bufs=2,
                     name=f"tp_{label}_{cc}_{mc}")
        tpv = tp[:mw, :].rearrange("p (x two) -> p x two", two=2)[:, :, 0]
        nc.tensor.transpose(tpv, fb[:, cc, msl].bitcast(E3), ident)
        nc.vector.tensor_copy(fT[:mw, mc, cc * P:(cc + 1) * P].bitcast(E3),
                              tpv)

    f2b, f2T, rneg2 = key_norms(fm2, "k2")
    f3b, f3T, rneg3 = key_norms(fm3, "k3")
    # Interleaved k2/k3 transpose work items, dripped into the jp=0 score
    # emission so they fill PE/DVE idle slots under the ACT-paced exp stream.
    tjobs = []
    for cc in range(CC):
        for mc in range(NMC):
            tjobs.append((f2T, f2b, "k2", cc, mc))
            tjobs.append((f3T, f3b, "k3", cc, mc))
    emit = [0]

    def drip_transposes(k):
        hi = min(emit[0] + k, len(tjobs))
        for i in range(emit[0], hi):
            transpose_one(*tjobs[i])
        emit[0] = hi

    return dict(ones128=ones128, qn=qn, drip=drip_transposes,
                mats=((2, f2b, f2T, rneg2), (3, f3b, f3T, rneg3)))


def _build_main(tc, sb, ps, out_ap, fm1, st):
    nc = tc.nc
    ones128 = st["ones128"]
    qn = st["qn"]
    mats = st["mats"]
    drip = st["drip"]

    # =======================================================================
    # main loop over n-chunk pairs: (0,1),(2,3),(4,5),(6,)
    # Scores for both chunks of a pair land in one 2-bank psum tile so a
    # single ACT exp (per-partition scale is identical) covers both.
    # =======================================================================
    NPAIR = NMC // 2  # 12 DoubleRow pairs + 1 tail chunk (64 rows)
    NJP = (NNC + 1) // 2

    def _npj(jp):
        return 2 if 2 * jp + 1 < NNC else 1

    def _jss(jp):
        return [slice((2 * jp + jj) * NW, (2 * jp + jj + 1) * NW)
                for jj in range(_npj(jp))]

    Es = {}  # (jp, mat) -> E tile

    def emit_scores(jp):
        npj = _npj(jp)
        jss = _jss(jp)
        for mat, fb, fT, rneg in mats:
            E = sb.tile([P, NMC, 2, NW], FP8, tag=f"E{mat}", bufs=2,
                        name=f"E{mat}_{jp}")
            Es[(jp, mat)] = E
            for mc in range(NMC):
                mw = _mw(mc)
                msl = slice(mc * P, mc * P + mw)
                # [128, 1024] spans 2 psum banks; halves at 0 and 512 so each
                # matmul output stays inside one bank
                sp = ps.tile([P, 1024], F32, tag="sp", bufs=2,
                             name=f"sp_{jp}_{mat}_{mc}")
                for i in range(CC // 2):
                    for jj in range(npj):
                        nc.tensor.matmul(sp[:mw, jj * 512:jj * 512 + NW],
                                         fb[:, 2 * i:2 * i + 2, msl],
                                         qn[:, 2 * i:2 * i + 2, jss[jj]],
                                         start=(i == 0),
                                         stop=(i == CC // 2 - 1),
                                         perf_mode=DR)
                spv = sp[:mw, :].rearrange("p (t x) -> p t x", t=2)
                nc.scalar.activation(E[:mw, mc, :npj, :], spv[:, :npj, :NW],
                                     mybir.ActivationFunctionType.Exp,
                                     bias=0.0, scale=rneg[:mw, mc:mc + 1])
                if jp == 0:
                    drip(4)
        if jp == 0:
            drip(1000)  # flush any remaining transpose jobs

    # software pipeline: scores for jp+1 are emitted (and thus PE-prioritized)
    # ahead of the value phase of jp, so the ACT exp stream never starves at
    # pair boundaries. E bufs=2 holds exactly two pairs in flight.
    emit_scores(0)
    for jp in range(NJP):
        if jp + 1 < NJP:
            emit_scores(jp + 1)
        npj = _npj(jp)
        jss = _jss(jp)
        for mat, fb, fT, rneg in mats:
            E = Es.pop((jp, mat))
            for jj in range(npj):
                js = jss[jj]
                # softmax denominator (broadcast over partitions)
                cs = ps.tile([P, NW], F32, tag="cs", bufs=2,
                             name=f"cs_{jp}_{jj}_{mat}")
                for i in range(NPAIR):
                    nc.tensor.matmul(cs, ones128, E[:, 2 * i:2 * i + 2, jj, :],
                                     start=(i == 0), stop=False, perf_mode=DR)
                nc.tensor.matmul(cs, ones128[:MTAIL, 0, :],
                                 E[:MTAIL, NMC - 1, jj, :],
                                 start=False, stop=True)
                rs = sb.tile([P, NW], F32, tag="rs", bufs=2,
                             name=f"rs_{jp}_{jj}_{mat}")
                nc.vector.reciprocal_approx_fast(rs, cs)
                nc.vector.tensor_scalar_mul(rs, rs, FACTOR)

                # values: nu[c, n] = sum_m fT[m, c] * E[m, n]
                for cc in range(CC):
                    csl = slice(cc * P, (cc + 1) * P)
                    vp = ps.tile([P, NW], F32, tag="vp", bufs=2,
                                 name=f"vp_{jp}_{jj}_{mat}_{cc}")
                    for i in range(NPAIR):
                        nc.tensor.matmul(vp, fT[:, 2 * i:2 * i + 2, csl],
                                         E[:, 2 * i:2 * i + 2, jj, :],
                                         start=(i == 0), stop=False,
                                         perf_mode=DR)
                    nc.tensor.matmul(vp, fT[:MTAIL, NMC - 1, csl],
                                     E[:MTAIL, NMC - 1, jj, :],
                                     start=False, stop=True)
                    tmp = sb.tile([P, NW], F32, tag="t", bufs=3,
                                  name=f"t_{jp}_{jj}_{mat}_{cc}")
                    nc.vector.tensor_mul(tmp, vp, rs)
                    if mat == 2:
                        # out = f1 + tmp2, streamed straight to DRAM
                        fs = sb.tile([P, NW], F32, tag="f1s", bufs=3,
                                     name=f"f1s_{jp}_{jj}_{cc}")
                        nc.sync.dma_start(fs, fm1[cc * P:(cc + 1) * P, js])
                        o = sb.tile([P, NW], F32, tag="outs", bufs=3,
                                    name=f"o_{jp}_{jj}_{cc}")
                        nc.vector.tensor_add(o, tmp, fs)
                        nc.sync.dma_start(out_ap[cc * P:(cc + 1) * P, js], o)
                    else:
                        # accumulate the mat3 contribution in DRAM via DMA
                        nc.gpsimd.dma_start(out_ap[cc * P:(cc + 1) * P, js],
                                            tmp,
                                            accum_op=mybir.AluOpType.add)


def _build():
    nc = bacc.Bacc("TRN2", target_bir_lowering=False, debug=False,
                   num_devices=B)
    fm1 = nc.dram_tensor("fm1", [C, HW], F32, kind="ExternalInput").ap()
    fm2 = nc.dram_tensor("fm2", [C, HW], F32, kind="ExternalInput").ap()
    fm3 = nc.dram_tensor("fm3", [C, HW], F32, kind="ExternalInput").ap()
    out = nc.dram_tensor("out", [C, HW], F32, kind="ExternalOutput").ap()

    with tile.TileContext(nc) as tc:
        with tc.tile_pool(name="sbP", bufs=1) as sbP, \
             tc.tile_pool(name="ps", bufs=1, space="PSUM") as ps:
            with tc.tile_pool(name="pre", bufs=1) as pre:
                st = _build_preproc(tc, sbP, pre, ps, fm1, fm2, fm3)
            with tc.tile_pool(name="sbm", bufs=1) as sbm:
                _build_main(tc, sbm, ps, out, fm1, st)
    nc.compile()
    return nc


def _get_nc():
    global _cached_nc
    if _cached_nc is None:
        _cached_nc = _build()
    return _cached_nc


def kernel(**inputs):
    fm1 = np.ascontiguousarray(
        np.asarray(inputs["fm1"], dtype=np.float32).reshape(B, C, HW))
    fm2 = np.ascontiguousarray(
        np.asarray(inputs["fm2"], dtype=np.float32).reshape(B, C, HW))
    fm3 = np.ascontiguousarray(
        np.asarray(inputs["fm3"], dtype=np.float32).reshape(B, C, HW))

    nc = _get_nc()
    in_maps = [{"fm1": fm1[b], "fm2": fm2[b], "fm3": fm3[b]} for b in range(B)]
    res = run_bass_kernel_spmd(nc, in_maps, core_ids=list(range(B)),
                               trace=TRACE)
    kernel.last_results = res
    out = np.stack([res.results[b]["out"] for b in range(B)])
    return out.reshape(B, C, H, W).astype(np.float32)


if __name__ == "__main__":
    rng = np.random.default_rng(0)
    ins = {k: rng.standard_normal((B, C, H, W)).astype(np.float32)
           for k in ("fm1", "fm2", "fm3")}
    o = kernel(**ins)
    print("out shape", o.shape, o.dtype)


# revision 49
# speedup vs baseline: 1.3986x; 1.3986x over previous
"""Trainium2 Bass kernel for nn_FDM_46394236731667.

Computes, per batch b (b = 0..7, one NeuronCore each):
    f1,f2,f3 = fm{1,2,3}[b].reshape(C, HW)
    qn  = f1 / max(||f1||_col, eps)  (column-wise L2 over channels)
    s_k = -(qn^T @ (f_k / max(||f_k||_col, eps)))          k in {2,3}
    a_k = softmax(s_k, axis=-1)
    out[b] = f1 + 0.001 * (f2 @ a2^T + f3 @ a3^T)

Implementation notes:
  - Scores are computed TRANSPOSED (tiles [m_partition, n_free]) so that the
    key-norm scale r_k[m] is a per-partition scalar folded into the ACT exp
    instruction (out = exp(scale[p] * psum)), the softmax denominator is a
    partition-reduction done with an all-ones matmul (result arrives already
    broadcast across partitions), and the P@V matmul consumes the exp tiles
    directly with no transposes of probability tiles.
  - Query norms r1[n] ride on the free axis, so they are pre-folded into the
    query operand qn = 16 * f1 * broadcast(r1) (the 16 makes fp8 quantization
    of unit-norm entries land in the normal range; the matching 1/16 is folded
    into the exp scale).
  - All three big matmul families (scores, softmax-denominator, values) run in
    fp8(e4m3) with perf_mode=DoubleRow: operands are stored with the
    contraction-chunk index as a middle AP dim so each matmul contracts 256
    rows. The output is fm1 + 0.001*(attention terms), so low-precision
    attention arithmetic perturbs the output by ~1e-6 relative.
  - Softmax skips the max-subtraction: scores are cosine similarities in
    [-1, 1], so exp() cannot overflow.
"""
import os
import sys

for _p in ("/opt/trn_rl_repo", "/root/.axon_site/_ro/trn_rl_repo"):
    if os.path.isdir(_p) and _p not in sys.path:
        sys.path.insert(0, _p)

import numpy as np

import concourse.bass as bass
import concourse.tile as tile
from concourse import bacc, mybir
from concourse.bass_utils import run_bass_kernel_spmd
from concourse.masks import make_identity

B, C, H, W = 8, 512, 56, 56
HW = H * W            # 3136
P = 128
CC = C // P           # 4 channel chunks
NMC = 25              # m chunks: 24 x 128 + 1 x 64
MTAIL = HW - 24 * P   # 64
NNC = 7               # n chunks
NW = HW // NNC        # 448
EPS = 1e-12
FACTOR = 0.001
QSCALE = 16.0         # fp8 headroom scale on qn; 1/QSCALE folded into exp

dt = mybir.dt
F32, BF16, FP8 = dt.float32, dt.bfloat16, dt.float8e4
DR = mybir.MatmulPerfMode.DoubleRow

TRACE = False
_cached_nc = None


def _mw(mc):
    return P if mc < NMC - 1 else MTAIL


def _build_preproc(tc, sbP, pre, ps, fm1, fm2, fm3):
    """sbP: persistent operand pool; pre: preproc transients (released before
    the main loop so its SBUF is reused for the main-loop pool)."""
    nc = tc.nc

    # ---- constants ----
    # fp8e5 identity: transposes run on e4m3 data bit-reinterpreted as e5m2
    # (pure data movement; our data never hits the e5m2 Inf/NaN encodings)
    ident = sbP.tile([P, P], dt.float8e3, tag="ident", name="ident")
    make_identity(nc, ident)
    ones128 = sbP.tile([P, 2, P], FP8, tag="ones128", name="ones128")
    nc.vector.memset(ones128, 1.0)
    ones_col = sbP.tile([P, 1], FP8, tag="ones_col", name="ones_col")
    nc.vector.memset(ones_col, 1.0)

    # =======================================================================
    # f1 phase: qn = QSCALE * f1 * broadcast(1/max(||f1||, eps)), fp8,
    # stored as [P, CC, HW] so score matmuls can take [P, 2, NW] slices.
    # =======================================================================
    # f1 squares are pre-scaled by 1/QSCALE so the resulting reciprocal norm
    # comes out as QSCALE/||f1|| with no extra pass.
    fsq1 = []
    fr1b = []
    for cc in range(CC):
        fr = pre.tile([P, HW], F32, tag="fraw", bufs=8, name=f"f1raw_{cc}")
        nc.sync.dma_start(fr, fm1[cc * P:(cc + 1) * P, :])
        t8 = pre.tile([P, HW], FP8, tag="fsq", bufs=4, name=f"fsq1_{cc}")
        nc.scalar.activation(t8, fr, mybir.ActivationFunctionType.Square,
                             bias=0.0, scale=1.0 / QSCALE)
        fsq1.append(t8)
        fr1b.append(fr)

    # rbf[p, n] = QSCALE/max(||f1[:,n]||, eps); qn produced per n-chunk so the
    # first score matmuls can start before the whole f1 phase finishes.
    rbf = pre.tile([P, HW], F32, tag="rbf", name="rbf")
    qn = sbP.tile([P, CC, HW], FP8, tag="qn", name="qn")
    for j in range(NNC):
        js = slice(j * NW, (j + 1) * NW)
        ssb = ps.tile([P, NW], F32, tag="cs", bufs=2, name=f"ss1b_{j}")
        for cc in range(CC):
            nc.tensor.matmul(ssb, ones128[:, 0, :], fsq1[cc][:, js],
                             start=(cc == 0), stop=(cc == CC - 1))
        ns = pre.tile([P, NW], F32, tag="rtmp", bufs=2, name=f"ns1_{j}")
        nc.scalar.sqrt(ns, ssb)
        nc.vector.tensor_scalar_max(ns, ns, EPS / QSCALE)
        nc.vector.reciprocal_approx_fast(rbf[:, js], ns)
        for cc in range(CC):
            nc.vector.tensor_mul(qn[:, cc, js], fr1b[cc][:, js], rbf[:, js])

    # =======================================================================
    # f2/f3 phases: fp8 key operand [P, CC, HW], fp8 transpose [P, NMC, C],
    # per-column key norms (as [P, NMC] columns for the exp scale)
    # =======================================================================
    def key_norms(fm, label):
        fb = sbP.tile([P, CC, HW], FP8, tag=f"{label}b", name=f"{label}b")
        fT = sbP.tile([P, NMC, C], FP8, tag=f"{label}T", name=f"{label}T")
        fsq = []
        for cc in range(CC):
            fr = pre.tile([P, HW], F32, tag="fraw", bufs=8,
                          name=f"{label}raw_{cc}")
            nc.sync.dma_start(fr, fm[cc * P:(cc + 1) * P, :])
            nc.vector.tensor_copy(fb[:, cc, :], fr)
            t8 = pre.tile([P, HW], FP8, tag="fsq", bufs=4,
                          name=f"{label}sq_{cc}")
            nc.scalar.square(t8, fr)
            fsq.append(t8)

        # ss columns: ssc[:mw, mc] = sum_c f[c, mc*128+p]^2
        ssc = ps.tile([P, NMC], F32, tag="cs", bufs=2, name=f"ssc_{label}")
        for mc in range(NMC):
            mw = _mw(mc)
            msl = slice(mc * P, mc * P + mw)
            for cc in range(CC):
                nc.tensor.matmul(ssc[:mw, mc:mc + 1], fsq[cc][:, msl], ones_col,
                                 start=(cc == 0), stop=(cc == CC - 1))
        nrm = pre.tile([P, NMC], F32, tag="rtmp2", bufs=2, name=f"nrm_{label}")
        nc.scalar.sqrt(nrm, ssc)
        nc.vector.tensor_scalar_max(nrm, nrm, EPS)
        rcp = pre.tile([P, NMC], F32, tag="rtmp3", bufs=2, name=f"rcp_{label}")
        nc.vector.reciprocal_approx_fast(rcp, nrm)
        rneg = sbP.tile([P, NMC], F32, tag=f"rneg_{label}", name=f"rneg_{label}")
        nc.vector.tensor_scalar_mul(rneg, rcp, -1.0 / QSCALE)
        return fb, fT, rneg

    E3 = dt.float8e3

    def transpose_one(fT, fb, label, cc, mc):
        # fT[p, mc, c] = f[c, mc*128+p]; PE transpose of the fp8 key operand
        # (bytes viewed as e5m2). The tp psum tiles share the "vp" tag: the
        # first value-matmul psum needs the transposes finished anyway, while
        # the score psum rotation ("sp" tag) stays free of them.
        mw = _mw(mc)
        msl = slice(mc * P, mc * P + mw)
        # fp8 transpose mode writes psum with element step 2
        tp = ps.tile([P, 2 * P], E3, tag="vp", bufs=2,
                     name=f"tp_{label}_{cc}_{mc}")
        tpv = tp[:mw, :].rearrange("p (x two) -> p x two", two=2)[:, :, 0]
        nc.tensor.transpose(tpv, fb[:, cc, msl].bitcast(E3), ident)
        nc.vector.tensor_copy(fT[:mw, mc, cc * P:(cc + 1) * P].bitcast(E3),
                              tpv)

    f2b, f2T, rneg2 = key_norms(fm2, "k2")
    f3b, f3T, rneg3 = key_norms(fm3, "k3")
    # Interleaved k2/k3 transpose work items, dripped into the jp=0 score
    # emission so they fill PE/DVE idle slots under the ACT-paced exp stream.
    tjobs = []
    for cc in range(CC):
        for mc in range(NMC):
            tjobs.append((f2T, f2b, "k2", cc, mc))
            tjobs.append((f3T, f3b, "k3", cc, mc))
    emit = [0]

    def drip_transposes(k):
        hi = min(emit[0] + k, len(tjobs))
        for i in range(emit[0], hi):
            transpose_one(*tjobs[i])
        emit[0] = hi

    return dict(ones128=ones128, qn=qn, drip=drip_transposes,
                mats=((2, f2b, f2T, rneg2), (3, f3b, f3T, rneg3)))


def _build_main(tc, sb, ps, out_ap, fm1, st):
    nc = tc.nc
    ones128 = st["ones128"]
    qn = st["qn"]
    mats = st["mats"]
    drip = st["drip"]

    # =======================================================================
    # main loop over n-chunk pairs: (0,1),(2,3),(4,5),(6,)
    # Scores for both chunks of a pair land in one 2-bank psum tile so a
    # single ACT exp (per-partition scale is identical) covers both.
    # =======================================================================
    NPAIR = NMC // 2  # 12 DoubleRow pairs + 1 tail chunk (64 rows)
    NJP = (NNC + 1) // 2

    def _npj(jp):
        return 2 if 2 * jp + 1 < NNC else 1

    def _jss(jp):
        return [slice((2 * jp + jj) * NW, (2 * jp + jj + 1) * NW)
                for jj in range(_npj(jp))]

    Es = {}  # (jp, mat) -> E tile

    def emit_scores(jp):
        npj = _npj(jp)
        jss = _jss(jp)
        for mat, fb, fT, rneg in mats:
            E = sb.tile([P, NMC, 2, NW], FP8, tag=f"E{mat}", bufs=2,
                        name=f"E{mat}_{jp}")
            Es[(jp, mat)] = E
            for mc in range(NMC):
                mw = _mw(mc)
                msl = slice(mc * P, mc * P + mw)
                # [128, 1024] spans 2 psum banks; halves at 0 and 512 so each
                # matmul output stays inside one bank
                sp = ps.tile([P, 1024], F32, tag="sp", bufs=2,
                             name=f"sp_{jp}_{mat}_{mc}")
                for i in range(CC // 2):
                    for jj in range(npj):
                        nc.tensor.matmul(sp[:mw, jj * 512:jj * 512 + NW],
                                         fb[:, 2 * i:2 * i + 2, msl],
                                         qn[:, 2 * i:2 * i + 2, jss[jj]],
                                         start=(i == 0),
                                         stop=(i == CC // 2 - 1),
                                         perf_mode=DR)
                spv = sp[:mw, :].rearrange("p (t x) -> p t x", t=2)
                nc.scalar.activation(E[:mw, mc, :npj, :], spv[:, :npj, :NW],
                                     mybir.ActivationFunctionType.Exp,
                                     bias=0.0, scale=rneg[:mw, mc:mc + 1])
                if jp == 0:
                    drip(4)
        if jp == 0:
            drip(1000)  # flush any remaining transpose jobs

    # software pipeline: scores for jp+1 are emitted (and thus PE-prioritized)
    # ahead of the value phase of jp, so the ACT exp stream never starves at
    # pair boundaries. E bufs=2 holds exactly two pairs in flight.
    emit_scores(0)
    for jp in range(NJP):
        if jp + 1 < NJP:
            emit_scores(jp + 1)
        npj = _npj(jp)
        jss = _jss(jp)
        for mat, fb, fT, rneg in mats:
            E = Es.pop((jp, mat))
            for jj in range(npj):
                js = jss[jj]
                # softmax denominator (broadcast over partitions)
                cs = ps.tile([P, NW], F32, tag="cs", bufs=2,
                             name=f"cs_{jp}_{jj}_{mat}")
                for i in range(NPAIR):
                    nc.tensor.matmul(cs, ones128, E[:, 2 * i:2 * i + 2, jj, :],
                                     start=(i == 0), stop=False, perf_mode=DR)
                nc.tensor.matmul(cs, ones128[:MTAIL, 0, :],
                                 E[:MTAIL, NMC - 1, jj, :],
                                 start=False, stop=True)
                rs = sb.tile([P, NW], F32, tag="rs", bufs=3,
                             name=f"rs_{jp}_{jj}_{mat}")
                nc.vector.reciprocal_approx_fast(rs, cs)
                nc.vector.tensor_scalar_mul(rs, rs, FACTOR)

                # values: nu[c, n] = sum_m fT[m, c] * E[m, n]
                for cc in range(CC):
                    csl = slice(cc * P, (cc + 1) * P)
                    vp = ps.tile([P, NW], F32, tag="vp", bufs=2,
                                 name=f"vp_{jp}_{jj}_{mat}_{cc}")
                    for i in range(NPAIR):
                        nc.tensor.matmul(vp, fT[:, 2 * i:2 * i + 2, csl],
                                         E[:, 2 * i:2 * i + 2, jj, :],
                                         start=(i == 0), stop=False,
                                         perf_mode=DR)
                    nc.tensor.matmul(vp, fT[:MTAIL, NMC - 1, csl],
                                     E[:MTAIL, NMC - 1, jj, :],
                                     start=False, stop=True)
                    tmp = sb.tile([P, NW], F32, tag="t", bufs=4,
                                  name=f"t_{jp}_{jj}_{mat}_{cc}")
                    nc.vector.tensor_mul(tmp, vp, rs)
                    if mat == 2:
                        # out = f1 + tmp2, streamed straight to DRAM
                        fs = sb.tile([P, NW], F32, tag="f1s", bufs=4,
                                     name=f"f1s_{jp}_{jj}_{cc}")
                        nc.sync.dma_start(fs, fm1[cc * P:(cc + 1) * P, js])
                        o = sb.tile([P, NW], F32, tag="outs", bufs=4,
                                    name=f"o_{jp}_{jj}_{cc}")
                        nc.vector.tensor_add(o, tmp, fs)
                        nc.sync.dma_start(out_ap[cc * P:(cc + 1) * P, js], o)
                    else:
                        # accumulate the mat3 contribution in DRAM via DMA
                        nc.gpsimd.dma_start(out_ap[cc * P:(cc + 1) * P, js],
                                            tmp,
                                            accum_op=mybir.AluOpType.add)


def _build():
    nc = bacc.Bacc("TRN2", target_bir_lowering=False, debug=False,
                   num_devices=B)
    fm1 = nc.dram_tensor("fm1", [C, HW], F32, kind="ExternalInput").ap()
    fm2 = nc.dram_tensor("fm2", [C, HW], F32, kind="ExternalInput").ap()
    fm3 = nc.dram_tensor("fm3", [C, HW], F32, kind="ExternalInput").ap()
    out = nc.dram_tensor("out", [C, HW], F32, kind="ExternalOutput").ap()

    with tile.TileContext(nc) as tc:
        with tc.tile_pool(name="sbP", bufs=1) as sbP, \
             tc.tile_pool(name="ps", bufs=1, space="PSUM") as ps:
            with tc.tile_pool(name="pre", bufs=1) as pre:
                st = _build_preproc(tc, sbP, pre, ps, fm1, fm2, fm3)
            with tc.tile_pool(name="sbm", bufs=1) as sbm:
                _build_main(tc, sbm, ps, out, fm1, st)
    nc.compile()
    return nc


def _get_nc():
    global _cached_nc
    if _cached_nc is None:
        _cached_nc = _build()
    return _cached_nc


def kernel(**inputs):
    fm1 = np.ascontiguousarray(
        np.asarray(inputs["fm1"], dtype=np.float32).reshape(B, C, HW))
    fm2 = np.ascontiguousarray(
        np.asarray(inputs["fm2"], dtype=np.float32).reshape(B, C, HW))
    fm3 = np.ascontiguousarray(
        np.asarray(inputs["fm3"], dtype=np.float32).reshape(B, C, HW))

    nc = _get_nc()
    in_maps = [{"fm1": fm1[b], "fm2": fm2[b], "fm3": fm3[b]} for b in range(B)]
    res = run_bass_kernel_spmd(nc, in_maps, core_ids=list(range(B)),
                               trace=TRACE)
    kernel.last_results = res
    out = np.stack([res.results[b]["out"] for b in range(B)])
    return out.reshape(B, C, H, W).astype(np.float32)


if __name__ == "__main__":
    rng = np.random.default_rng(0)
    ins = {k: rng.standard_normal((B, C, H, W)).astype(np.float32)
           for k in ("fm1", "fm2", "fm3")}
    o = kernel(**ins)
    print("out shape", o.shape, o.dtype)


# revision 50
# speedup vs baseline: 11.9754x; 8.5626x over previous
"""Trainium2 Bass kernel for nn_FDM_46394236731667.

Computes, per batch b (b = 0..7, one NeuronCore each):
    f1,f2,f3 = fm{1,2,3}[b].reshape(C, HW)
    qn  = f1 / max(||f1||_col, eps)  (column-wise L2 over channels)
    s_k = -(qn^T @ (f_k / max(||f_k||_col, eps)))          k in {2,3}
    a_k = softmax(s_k, axis=-1)
    out[b] = f1 + 0.001 * (f2 @ a2^T + f3 @ a3^T)

Implementation notes:
  - Scores are computed TRANSPOSED (tiles [m_partition, n_free]) so that the
    key-norm scale r_k[m] is a per-partition scalar folded into the ACT exp
    instruction (out = exp(scale[p] * psum)), the softmax denominator is a
    partition-reduction done with an all-ones matmul (result arrives already
    broadcast across partitions), and the P@V matmul consumes the exp tiles
    directly with no transposes of probability tiles.
  - Query norms r1[n] ride on the free axis, so they are pre-folded into the
    query operand qn = 16 * f1 * broadcast(r1) (the 16 makes fp8 quantization
    of unit-norm entries land in the normal range; the matching 1/16 is folded
    into the exp scale).
  - All three big matmul families (scores, softmax-denominator, values) run in
    fp8(e4m3) with perf_mode=DoubleRow: operands are stored with the
    contraction-chunk index as a middle AP dim so each matmul contracts 256
    rows. The output is fm1 + 0.001*(attention terms), so low-precision
    attention arithmetic perturbs the output by ~1e-6 relative.
  - Softmax skips the max-subtraction: scores are cosine similarities in
    [-1, 1], so exp() cannot overflow.
"""
import os
import sys

for _p in ("/opt/trn_rl_repo", "/root/.axon_site/_ro/trn_rl_repo"):
    if os.path.isdir(_p) and _p not in sys.path:
        sys.path.insert(0, _p)

import numpy as np

import concourse.bass as bass
import concourse.tile as tile
from concourse import bacc, mybir
from concourse.bass_utils import run_bass_kernel_spmd
from concourse.masks import make_identity

B, C, H, W = 8, 512, 56, 56
HW = H * W            # 3136
P = 128
CC = C // P           # 4 channel chunks
NMC = 25              # m chunks: 24 x 128 + 1 x 64
MTAIL = HW - 24 * P   # 64
NNC = 7               # n chunks
NW = HW // NNC        # 448
EPS = 1e-12
FACTOR = 0.001
QSCALE = 16.0         # fp8 headroom scale on qn; 1/QSCALE folded into exp

dt = mybir.dt
F32, BF16, FP8 = dt.float32, dt.bfloat16, dt.float8e4
DR = mybir.MatmulPerfMode.DoubleRow

TRACE = False
_cached_nc = None


def _mw(mc):
    return P if mc < NMC - 1 else MTAIL


def _build_preproc(tc, sbP, pre, ps, fm1, fm2, fm3):
    """sbP: persistent operand pool; pre: preproc transients (released before
    the main loop so its SBUF is reused for the main-loop pool)."""
    nc = tc.nc

    # ---- constants ----
    # fp8e5 identity: transposes run on e4m3 data bit-reinterpreted as e5m2
    # (pure data movement; our data never hits the e5m2 Inf/NaN encodings)
    ident = sbP.tile([P, P], dt.float8e3, tag="ident", name="ident")
    make_identity(nc, ident)
    ones128 = sbP.tile([P, 2, P], FP8, tag="ones128", name="ones128")
    nc.vector.memset(ones128, 1.0)
    ones_col = sbP.tile([P, 1], FP8, tag="ones_col", name="ones_col")
    nc.vector.memset(ones_col, 1.0)

    # =======================================================================
    # f1 phase: qn = QSCALE * f1 * broadcast(1/max(||f1||, eps)), fp8,
    # stored as [P, CC, HW] so score matmuls can take [P, 2, NW] slices.
    # =======================================================================
    # f1 squares are pre-scaled by 1/QSCALE so the resulting reciprocal norm
    # comes out as QSCALE/||f1|| with no extra pass.
    fsq1 = []
    fr1b = []
    for cc in range(CC):
        fr = pre.tile([P, HW], F32, tag="fraw", bufs=8, name=f"f1raw_{cc}")
        nc.sync.dma_start(fr, fm1[cc * P:(cc + 1) * P, :])
        t8 = pre.tile([P, HW], FP8, tag="fsq", bufs=8, name=f"fsq1_{cc}")
        nc.scalar.activation(t8, fr, mybir.ActivationFunctionType.Square,
                             bias=0.0, scale=1.0 / QSCALE)
        fsq1.append(t8)
        fr1b.append(fr)

    # rbf[p, n] = QSCALE/max(||f1[:,n]||, eps); qn produced per n-chunk so the
    # first score matmuls can start before the whole f1 phase finishes.
    rbf = pre.tile([P, HW], F32, tag="rbf", name="rbf")
    qn = sbP.tile([P, CC, HW], FP8, tag="qn", name="qn")
    for j in range(NNC):
        js = slice(j * NW, (j + 1) * NW)
        ssb = ps.tile([P, NW], F32, tag="cs", bufs=2, name=f"ss1b_{j}")
        for cc in range(CC):
            nc.tensor.matmul(ssb, ones128[:, 0, :], fsq1[cc][:, js],
                             start=(cc == 0), stop=(cc == CC - 1))
        ns = pre.tile([P, NW], F32, tag="rtmp", bufs=2, name=f"ns1_{j}")
        nc.scalar.sqrt(ns, ssb)
        nc.vector.tensor_scalar_max(ns, ns, EPS / QSCALE)
        nc.vector.reciprocal_approx_fast(rbf[:, js], ns)
        for cc in range(CC):
            nc.vector.tensor_mul(qn[:, cc, js], fr1b[cc][:, js], rbf[:, js])

    # =======================================================================
    # f2/f3 phases: fp8 key operand [P, CC, HW], fp8 transpose [P, NMC, C],
    # per-column key norms (as [P, NMC] columns for the exp scale)
    # =======================================================================
    def key_norms(fm, label):
        fb = sbP.tile([P, CC, HW], FP8, tag=f"{label}b", name=f"{label}b")
        fT = sbP.tile([P, NMC, C], FP8, tag=f"{label}T", name=f"{label}T")
        fsq = []
        for cc in range(CC):
            fr = pre.tile([P, HW], F32, tag="fraw", bufs=8,
                          name=f"{label}raw_{cc}")
            nc.sync.dma_start(fr, fm[cc * P:(cc + 1) * P, :])
            nc.vector.tensor_copy(fb[:, cc, :], fr)
            t8 = pre.tile([P, HW], FP8, tag="fsq", bufs=8,
                          name=f"{label}sq_{cc}")
            nc.scalar.square(t8, fr)
            fsq.append(t8)

        # ss columns: ssc[:mw, mc] = sum_c f[c, mc*128+p]^2
        ssc = ps.tile([P, NMC], F32, tag="cs", bufs=2, name=f"ssc_{label}")
        for mc in range(NMC):
            mw = _mw(mc)
            msl = slice(mc * P, mc * P + mw)
            for cc in range(CC):
                nc.tensor.matmul(ssc[:mw, mc:mc + 1], fsq[cc][:, msl], ones_col,
                                 start=(cc == 0), stop=(cc == CC - 1))
        nrm = pre.tile([P, NMC], F32, tag="rtmp2", bufs=2, name=f"nrm_{label}")
        nc.scalar.sqrt(nrm, ssc)
        nc.vector.tensor_scalar_max(nrm, nrm, EPS)
        rcp = pre.tile([P, NMC], F32, tag="rtmp3", bufs=2, name=f"rcp_{label}")
        nc.vector.reciprocal_approx_fast(rcp, nrm)
        rneg = sbP.tile([P, NMC], F32, tag=f"rneg_{label}", name=f"rneg_{label}")
        nc.vector.tensor_scalar_mul(rneg, rcp, -1.0 / QSCALE)
        return fb, fT, rneg

    E3 = dt.float8e3

    def transpose_one(fT, fb, label, cc, mc):
        # fT[p, mc, c] = f[c, mc*128+p]; PE transpose of the fp8 key operand
        # (bytes viewed as e5m2). The tp psum tiles share the "vp" tag: the
        # first value-matmul psum needs the transposes finished anyway, while
        # the score psum rotation ("sp" tag) stays free of them.
        mw = _mw(mc)
        msl = slice(mc * P, mc * P + mw)
        # fp8 transpose mode writes psum with element step 2
        tp = ps.tile([P, 2 * P], E3, tag="vp", bufs=2,
                     name=f"tp_{label}_{cc}_{mc}")
        tpv = tp[:mw, :].rearrange("p (x two) -> p x two", two=2)[:, :, 0]
        nc.tensor.transpose(tpv, fb[:, cc, msl].bitcast(E3), ident)
        nc.vector.tensor_copy(fT[:mw, mc, cc * P:(cc + 1) * P].bitcast(E3),
                              tpv)

    f2b, f2T, rneg2 = key_norms(fm2, "k2")
    f3b, f3T, rneg3 = key_norms(fm3, "k3")
    # Interleaved k2/k3 transpose work items, dripped into the jp=0 score
    # emission so they fill PE/DVE idle slots under the ACT-paced exp stream.
    tjobs = []
    for cc in range(CC):
        for mc in range(NMC):
            tjobs.append((f2T, f2b, "k2", cc, mc))
            tjobs.append((f3T, f3b, "k3", cc, mc))
    emit = [0]

    def drip_transposes(k):
        hi = min(emit[0] + k, len(tjobs))
        for i in range(emit[0], hi):
            transpose_one(*tjobs[i])
        emit[0] = hi

    return dict(ones128=ones128, qn=qn, drip=drip_transposes,
                mats=((2, f2b, f2T, rneg2), (3, f3b, f3T, rneg3)))


def _build_main(tc, sb, ps, out_ap, fm1, st):
    nc = tc.nc
    ones128 = st["ones128"]
    qn = st["qn"]
    mats = st["mats"]
    drip = st["drip"]

    # =======================================================================
    # main loop over n-chunk pairs: (0,1),(2,3),(4,5),(6,)
    # Scores for both chunks of a pair land in one 2-bank psum tile so a
    # single ACT exp (per-partition scale is identical) covers both.
    # =======================================================================
    NPAIR = NMC // 2  # 12 DoubleRow pairs + 1 tail chunk (64 rows)
    NJP = (NNC + 1) // 2

    def _npj(jp):
        return 2 if 2 * jp + 1 < NNC else 1

    def _jss(jp):
        return [slice((2 * jp + jj) * NW, (2 * jp + jj + 1) * NW)
                for jj in range(_npj(jp))]

    Es = {}  # (jp, mat) -> E tile

    def emit_scores(jp):
        npj = _npj(jp)
        jss = _jss(jp)
        for mat, fb, fT, rneg in mats:
            E = sb.tile([P, NMC, 2, NW], FP8, tag=f"E{mat}", bufs=2,
                        name=f"E{mat}_{jp}")
            Es[(jp, mat)] = E
            for mc in range(NMC):
                mw = _mw(mc)
                msl = slice(mc * P, mc * P + mw)
                # [128, 1024] spans 2 psum banks; halves at 0 and 512 so each
                # matmul output stays inside one bank
                sp = ps.tile([P, 1024], F32, tag="sp", bufs=2,
                             name=f"sp_{jp}_{mat}_{mc}")
                for i in range(CC // 2):
                    for jj in range(npj):
                        nc.tensor.matmul(sp[:mw, jj * 512:jj * 512 + NW],
                                         fb[:, 2 * i:2 * i + 2, msl],
                                         qn[:, 2 * i:2 * i + 2, jss[jj]],
                                         start=(i == 0),
                                         stop=(i == CC // 2 - 1),
                                         perf_mode=DR)
                spv = sp[:mw, :].rearrange("p (t x) -> p t x", t=2)
                nc.scalar.activation(E[:mw, mc, :npj, :], spv[:, :npj, :NW],
                                     mybir.ActivationFunctionType.Exp,
                                     bias=0.0, scale=rneg[:mw, mc:mc + 1])
                if jp == 0:
                    drip(4)
        if jp == 0:
            drip(1000)  # flush any remaining transpose jobs

    # software pipeline: scores for jp+1 are emitted (and thus PE-prioritized)
    # ahead of the value phase of jp, so the ACT exp stream never starves at
    # pair boundaries. E bufs=2 holds exactly two pairs in flight.
    emit_scores(0)
    for jp in range(NJP):
        if jp + 1 < NJP:
            emit_scores(jp + 1)
        npj = _npj(jp)
        jss = _jss(jp)
        for mat, fb, fT, rneg in mats:
            E = Es.pop((jp, mat))
            for jj in range(npj):
                js = jss[jj]
                # softmax denominator (broadcast over partitions)
                cs = ps.tile([P, NW], F32, tag="cs", bufs=2,
                             name=f"cs_{jp}_{jj}_{mat}")
                for i in range(NPAIR):
                    nc.tensor.matmul(cs, ones128, E[:, 2 * i:2 * i + 2, jj, :],
                                     start=(i == 0), stop=False, perf_mode=DR)
                nc.tensor.matmul(cs, ones128[:MTAIL, 0, :],
                                 E[:MTAIL, NMC - 1, jj, :],
                                 start=False, stop=True)
                rs = sb.tile([P, NW], F32, tag="rs", bufs=3,
                             name=f"rs_{jp}_{jj}_{mat}")
                nc.vector.reciprocal_approx_fast(rs, cs)
                nc.vector.tensor_scalar_mul(rs, rs, FACTOR)

                # values: nu[c, n] = sum_m fT[m, c] * E[m, n]
                for cc in range(CC):
                    csl = slice(cc * P, (cc + 1) * P)
                    vp = ps.tile([P, NW], F32, tag="vp", bufs=2,
                                 name=f"vp_{jp}_{jj}_{mat}_{cc}")
                    for i in range(NPAIR):
                        nc.tensor.matmul(vp, fT[:, 2 * i:2 * i + 2, csl],
                                         E[:, 2 * i:2 * i + 2, jj, :],
                                         start=(i == 0), stop=False,
                                         perf_mode=DR)
                    nc.tensor.matmul(vp, fT[:MTAIL, NMC - 1, csl],
                                     E[:MTAIL, NMC - 1, jj, :],
                                     start=False, stop=True)
                    tmp = sb.tile([P, NW], F32, tag="t", bufs=4,
                                  name=f"t_{jp}_{jj}_{mat}_{cc}")
                    nc.vector.tensor_mul(tmp, vp, rs)
                    if mat == 2:
                        # out = f1 + tmp2, streamed straight to DRAM
                        fs = sb.tile([P, NW], F32, tag="f1s", bufs=4,
                                     name=f"f1s_{jp}_{jj}_{cc}")
                        nc.sync.dma_start(fs, fm1[cc * P:(cc + 1) * P, js])
                        o = sb.tile([P, NW], F32, tag="outs", bufs=4,
                                    name=f"o_{jp}_{jj}_{cc}")
                        nc.vector.tensor_add(o, tmp, fs)
                        nc.sync.dma_start(out_ap[cc * P:(cc + 1) * P, js], o)
                    else:
                        # accumulate the mat3 contribution in DRAM via DMA
                        nc.gpsimd.dma_start(out_ap[cc * P:(cc + 1) * P, js],
                                            tmp,
                                            accum_op=mybir.AluOpType.add)


def _build():
    nc = bacc.Bacc("TRN2", target_bir_lowering=False, debug=False,
                   num_devices=B)
    fm1 = nc.dram_tensor("fm1", [C, HW], F32, kind="ExternalInput").ap()
    fm2 = nc.dram_tensor("fm2", [C, HW], F32, kind="ExternalInput").ap()
    fm3 = nc.dram_tensor("fm3", [C, HW], F32, kind="ExternalInput").ap()
    out = nc.dram_tensor("out", [C, HW], F32, kind="ExternalOutput").ap()

    with tile.TileContext(nc) as tc:
        with tc.tile_pool(name="sbP", bufs=1) as sbP, \
             tc.tile_pool(name="ps", bufs=1, space="PSUM") as ps:
            with tc.tile_pool(name="pre", bufs=1) as pre:
                st = _build_preproc(tc, sbP, pre, ps, fm1, fm2, fm3)
            with tc.tile_pool(name="sbm", bufs=1) as sbm:
                _build_main(tc, sbm, ps, out, fm1, st)
    nc.compile()
    return nc


def _get_nc():
    global _cached_nc
    if _cached_nc is None:
        _cached_nc = _build()
    return _cached_nc


def kernel(**inputs):
    fm1 = np.ascontiguousarray(
        np.asarray(inputs["fm1"], dtype=np.float32).reshape(B, C, HW))
    fm2 = np.ascontiguousarray(
        np.asarray(inputs["fm2"], dtype=np.float32).reshape(B, C, HW))
    fm3 = np.ascontiguousarray(
        np.asarray(inputs["fm3"], dtype=np.float32).reshape(B, C, HW))

    nc = _get_nc()
    in_maps = [{"fm1": fm1[b], "fm2": fm2[b], "fm3": fm3[b]} for b in range(B)]
    res = run_bass_kernel_spmd(nc, in_maps, core_ids=list(range(B)),
                               trace=TRACE)
    kernel.last_results = res
    out = np.stack([res.results[b]["out"] for b in range(B)])
    return out.reshape(B, C, H, W).astype(np.float32)


if __name__ == "__main__":
    rng = np.random.default_rng(0)
    ins = {k: rng.standard_normal((B, C, H, W)).astype(np.float32)
           for k in ("fm1", "fm2", "fm3")}
    o = kernel(**ins)
    print("out shape", o.shape, o.dtype)
